# revision 10
# baseline (speedup 1.0000x reference)
"""Trainium2 Bass kernel for nn_CNN2D_48644799595070 (dynamic conv + attention + KAN).

Contract: kernel(**inputs) takes FULL unsharded inputs (np arrays keyed as in
setup_inputs) and returns the FULL [8192, 64] float32 output.  Internally:
batch is sharded over 8 NeuronCores (data parallel); all parameters are
replicated and host-folded into matmul-friendly fp16 tiles.

Math notes (device):
  conv:   1D Winograd F(2,3) along width.  Host ships V = x transformed per
          row (same byte count as x); device contracts 6 K-chunks (3 kh x 2
          cc) per (oh, t, och) Winograd-domain output: 96 matmuls per
          512-sample tile vs 144 direct.  Inverse transform (t -> ow,
          A = [[1,1,1,0],[0,1,-1,-1]]) is folded into the combine matmul.
  attn:   one-shot pre-pass over the whole core batch using a host-shipped
          GAP tile: fc1 -> relu -> fc2 -> exp(/T) -> recip-sum -> attn
          [4,B]; expanded to (kloc,oc) partitions via tiny matmuls and an
          ACT copy -> attnE fp16 in SBUF.
  combine: per (oh,t,och): ysb = ACT copy (PSUM->SBUF fp16); tmp = ysb *
          attnE (DVE fp16 2x); featP[oh] += A1sel[t]^T @ tmp (PSUM accum),
          + bias via attn16 matmul.
  KAN:    exact truncated-power spline: m16/mp16 = min/max(feat,0) fp16;
          r_q = (m16 - G_q) clamped (DVE dual-scalar TS, fp32 out); r2 =
          ACT Square batched 6 planes -> fp16; s_q = STT(m16,-G_q,r2) ->
          fp16; 24 plane matmuls + indicator + base silu + const row into
          one PSUM.
"""
import sys
sys.path.insert(0, "/opt/trn_rl_repo")

import numpy as np
from math import comb
from contextlib import ExitStack

import concourse.bass as bass
import concourse.tile as tile
from concourse import bacc, mybir
from concourse import bass_utils

# ---- problem constants (hardcoded per contract) ----
B_FULL = 8192
N_CORES = 8
B_CORE = B_FULL // N_CORES        # 1024
CIN = 256
COUT = 64
NK = 4
HIDDEN = 64
TEMP = 34.0
GRID_SIZE, SPLINE_ORDER = 5, 3
H = 0.4
G64 = np.arange(-SPLINE_ORDER, GRID_SIZE + SPLINE_ORDER + 1, dtype=np.float64) * H - 1.0
G32 = G64.astype(np.float32)

NT = 512          # b-tile (matmul moving free dim)
NTILES = B_CORE // NT

F32 = mybir.dt.float32
F16 = mybir.dt.float16
AF = mybir.ActivationFunctionType
ALU = mybir.AluOpType

# Winograd F(2,3) matrices
BT_W = np.array([[1, 0, -1, 0], [0, 1, 1, 0], [0, -1, 1, 0], [0, 1, 0, -1]], np.float32)
G1_W = np.array([[1, 0, 0], [.5, .5, .5], [.5, -.5, .5], [0, 0, 1]], np.float32)
A1_W = np.array([[1, 1, 1, 0], [0, 1, -1, -1]], np.float32)   # [ow, t]

_cached = {}


# --------------------------------------------------------------------------
# host-side weight folding
# --------------------------------------------------------------------------
def prepare_weights(weight, bias, fc1_w, fc1_b, fc2_w, fc2_b,
                    kan_base_w, kan_spline_w, kan_spline_scaler):
    """Fold all parameters into device tile layouts. Returns dict of np arrays."""
    d = {}
    f16 = np.float16
    # conv weights, Winograd domain: U[n,o,c,kh,t] = sum_kw G1[t,kw] w[n,o,c,kh,kw]
    w = np.asarray(weight, np.float32)                   # [NK, COUT, CIN, 3, 3]
    U = np.einsum("tw,nochw->nocht", G1_W, w).astype(f16)  # [NK,COUT,CIN,3,4]
    # lhsT tile per (t, kh, cc, och): [128 ci, 128 m=(kloc*64+oc)], k=och*2+kloc
    convW = np.empty((128, 4, 3, 2, 2, 128), f16)
    for t in range(4):
        for kh in range(3):
            for cc in range(2):
                for och in range(2):
                    blk = U[och * 2:och * 2 + 2, :, cc * 128:(cc + 1) * 128, kh, t]
                    # blk [2(kloc), 64(oc), 128(ci)] -> [ci, kloc*64+oc]
                    convW[:, t, kh, cc, och, :] = blk.reshape(128, 128).T
    d["convW"] = convW
    # combine lhsT per t: [128 (kloc,oc), 128 (ow*64+oc)] = A1[ow,t] * delta_oc
    A1sel = np.zeros((128, 4, 128), f16)
    for t in range(4):
        for p in range(128):
            oc = p % 64
            for ow in range(2):
                A1sel[p, t, ow * 64 + oc] = A1_W[ow, t]
    d["A1sel"] = A1sel
    # bias lhsT: [4 k, 128 (ow*64+oc)] = bias[k, oc]
    bW = np.zeros((4, 128), f16)
    bb = np.asarray(bias, np.float32)
    for k in range(4):
        for ow in range(2):
            bW[k, ow * 64:(ow + 1) * 64] = bb[k].astype(f16)
    d["biasW"] = bW
    # attention
    fc1 = np.asarray(fc1_w, np.float32)                  # xgap is the mean
    d["fc1"] = np.stack([fc1[:, cc * 128:(cc + 1) * 128].T.astype(f16)
                         for cc in range(2)], axis=1)    # [128, 2, 64]
    d["fc1b"] = np.asarray(fc1_b, np.float32).reshape(HIDDEN, 1)
    d["fc2"] = np.asarray(fc2_w, np.float32).T.astype(f16)    # [64, 4]
    d["fc2b34"] = (np.asarray(fc2_b, np.float32) / TEMP).reshape(NK, 1)
    E01 = np.zeros((4, 128), f16)
    E23 = np.zeros((4, 128), f16)
    for m in range(128):
        E01[m // 64, m] = 1.0
        E23[2 + m // 64, m] = 1.0
    d["E01"], d["E23"] = E01, E23

    # ---- KAN folding ----
    # device feat index i_new = po*64 + oc ; ref i = oc*4 + po
    i_new = np.arange(256)
    perm = (i_new % 64) * 4 + (i_new // 64)
    W2 = (np.asarray(kan_spline_w, np.float64)
          * np.asarray(kan_spline_scaler, np.float64)[..., None])   # [COUT,256,8]
    W2 = W2[:, perm, :]
    kbw = np.asarray(kan_base_w, np.float64)[:, perm]               # [COUT,256]

    c4 = np.array([comb(4, m) * (-1) ** m for m in range(5)], np.float64) / (6 * H ** 3)
    dd = np.zeros((COUT, 256, 12)); dp = np.zeros((COUT, 256, 12))
    for j in range(8):
        for m in range(5):
            dd[:, :, j + m] += W2[:, :, j] * c4[m]
            dp[:, :, j + 4 - m] += W2[:, :, j] * c4[m]
    A_L64 = dd[:, :, :6]           # left planes: s = (m-Gq)*relu(m-Gq)^2
    A_R64 = -dp[:, :, 6:]          # right planes: s = (mp-Gq)*relu(Gq-mp)^2 (= -r^3)

    def feedback_quant(A):
        Aq = np.empty_like(A)
        err = np.zeros(A.shape[:2])
        for q in range(A.shape[2]):
            t = A[:, :, q] + err
            Aq[:, :, q] = t.astype(f16).astype(np.float64)
            err = t - Aq[:, :, q]
        return Aq
    AL = feedback_quant(A_L64)
    AR = feedback_quant(A_R64[:, :, ::-1])[:, :, ::-1]

    # device-exact inactive plane values with the new op chain:
    #   left q (x>=0, m16=0):  r2 = f16(Gq^2); s = f16((0-Gq)*r2)
    #   right q (x<0, mp16=0): r2 = f16(Gq^2); s = f16((0-Gq)*r2)
    vL = np.empty(6, np.float64); wR = np.empty(6, np.float64)
    for q in range(6):
        r2 = np.float16(np.float32(-G32[q]) * np.float32(-G32[q]))
        vL[q] = np.float64(np.float16(np.float32(-G32[q]) * np.float32(r2)))
    for q in range(6, 12):
        r2 = np.float16(np.float32(-G32[q]) * np.float32(-G32[q]))
        wR[q - 6] = np.float64(np.float16(np.float32(-G32[q]) * np.float32(r2)))
    CposI = np.einsum("oiq,q->oi", AL, vL)
    CnegI = np.einsum("oiq,q->oi", AR, wR)
    Aind = (-(CnegI - CposI)).astype(f16)
    Cones = -CposI.sum(1)                      # [COUT] float64

    At = np.empty((128, 12, 2, 64), f16)       # [i_loc, q, ic, o]
    for q in range(12):
        srcq = AL[:, :, q] if q < 6 else AR[:, :, q - 6]   # [COUT, 256]
        for ic in range(2):
            At[:, q, ic, :] = srcq[:, ic * 128:(ic + 1) * 128].T.astype(f16)
    d["At"] = At
    d["Aind"] = np.stack([Aind[:, ic * 128:(ic + 1) * 128].T for ic in range(2)],
                         axis=1)                      # [128, 2, 64]
    d["baseW"] = np.stack([kbw[:, ic * 128:(ic + 1) * 128].T.astype(f16)
                           for ic in range(2)], axis=1)  # [128, 2, 64]
    C0hi = Cones.astype(f16)
    C0lo = (Cones - C0hi.astype(np.float64)).astype(f16)
    d["C0row"] = np.stack([C0hi, C0lo])          # [2, COUT] fp16
    return d


# --------------------------------------------------------------------------
# device kernel
# --------------------------------------------------------------------------
def build_nc(reps=1):
    nc = bacc.Bacc("TRN2", target_bir_lowering=False, debug=False,
                   enable_asserts=False, num_devices=N_CORES)
    dram = {}
    def din(name, shape, dt=F16):
        dram[name] = nc.dram_tensor(name, list(shape), dt, kind="ExternalInput").ap()
    din("x_t", (2, 128, 4, 4, B_CORE), F16)       # [cc, ci, t, row, b]
    din("xgap", (2, 128, B_CORE), F16)
    din("convW", (128, 4, 3, 2, 2, 128))
    din("A1sel", (128, 4, 128)); din("biasW", (4, 128))
    din("fc1", (128, 2, HIDDEN)); din("fc1b", (HIDDEN, 1), F32)
    din("fc2", (HIDDEN, NK)); din("fc2b34", (NK, 1), F32)
    din("E01", (4, 128)); din("E23", (4, 128))
    din("At", (128, 12, 2, COUT)); din("Aind", (128, 2, COUT))
    din("baseW", (128, 2, COUT)); din("C0row", (2, COUT))
    out = nc.dram_tensor("out", [COUT, B_CORE], F32, kind="ExternalOutput").ap()

    with tile.TileContext(nc) as tc, ExitStack() as ctx:
        wpool = ctx.enter_context(tc.tile_pool(name="weights", bufs=1))
        apool = ctx.enter_context(tc.tile_pool(name="attn", bufs=2))
        ppool = ctx.enter_context(tc.tile_pool(name="prepass", bufs=1))
        xpool = ctx.enter_context(tc.tile_pool(name="xdata", bufs=1))
        ypool = ctx.enter_context(tc.tile_pool(name="yevict", bufs=3))
        work = ctx.enter_context(tc.tile_pool(name="work", bufs=1))
        rpool = ctx.enter_context(tc.tile_pool(name="rpool", bufs=1))
        spool = ctx.enter_context(tc.tile_pool(name="splanes", bufs=1))
        opool = ctx.enter_context(tc.tile_pool(name="outbuf", bufs=2))
        ps_y = ctx.enter_context(tc.tile_pool(name="ps_y", bufs=3, space="PSUM"))
        ps_f = ctx.enter_context(tc.tile_pool(name="ps_feat", bufs=2, space="PSUM"))
        ps_o = ctx.enter_context(tc.tile_pool(name="ps_out", bufs=1, space="PSUM"))
        ps_at = ctx.enter_context(tc.tile_pool(name="ps_attn", bufs=1, space="PSUM"))

        # ---- DMA: attention inputs + weights first, then x tile 0 ----
        xgap = wpool.tile([128, 2, B_CORE], F16, name="xgap")
        nc.sync.dma_start(xgap[:, 0, :], dram["xgap"][0])
        nc.sync.dma_start(xgap[:, 1, :], dram["xgap"][1])
        fc1 = wpool.tile([128, 2, HIDDEN], F16); nc.sync.dma_start(fc1[:], dram["fc1"])
        fc1b = wpool.tile([HIDDEN, 1], F32); nc.sync.dma_start(fc1b[:], dram["fc1b"])
        fc2 = wpool.tile([HIDDEN, NK], F16); nc.sync.dma_start(fc2[:], dram["fc2"])
        fc2b = wpool.tile([NK, 1], F32); nc.sync.dma_start(fc2b[:], dram["fc2b34"])
        E01 = wpool.tile([4, 128], F16); nc.sync.dma_start(E01[:], dram["E01"])
        E23 = wpool.tile([4, 128], F16); nc.sync.dma_start(E23[:], dram["E23"])
        ones4 = wpool.tile([4, 128], F16); nc.any.memset(ones4[:], 1.0)
        ones2 = wpool.tile([2, NT], F16); nc.any.memset(ones2[:], 1.0)

        convW = wpool.tile([128, 4, 3, 2, 2, 128], F16)
        x_sb = []
        for cc in range(2):
            xc = xpool.tile([128, 4, 4, B_CORE], F16, tag=f"x{cc}", name=f"x_sb{cc}")
            x_sb.append(xc)
        # tile 0 x data + convW interleaved by t so conv can start early
        for t in range(4):
            nc.sync.dma_start(convW[:, t], dram["convW"][:, t])
            for cc in range(2):
                nc.sync.dma_start(x_sb[cc][:, t, :, 0:NT],
                                  dram["x_t"][cc, :, t, :, 0:NT])
        A1sel = wpool.tile([128, 4, 128], F16); nc.sync.dma_start(A1sel[:], dram["A1sel"])
        biasW = wpool.tile([4, 128], F16); nc.sync.dma_start(biasW[:], dram["biasW"])
        At = wpool.tile([128, 12, 2, COUT], F16); nc.sync.dma_start(At[:], dram["At"])
        Aind = wpool.tile([128, 2, COUT], F16); nc.sync.dma_start(Aind[:], dram["Aind"])
        baseW = wpool.tile([128, 2, COUT], F16); nc.sync.dma_start(baseW[:], dram["baseW"])
        C0row = wpool.tile([2, COUT], F16); nc.sync.dma_start(C0row[:], dram["C0row"])

        for _rep in range(reps):
            # per-rep attention outputs (double-buffered across reps)
            attnE = apool.tile([128, 2, B_CORE], F16, tag="attnE", name="attnE")
            attn16 = apool.tile([NK, B_CORE], F16, tag="attn16", name="attn16")
            # ---- attention pre-pass (full core batch, chunks of 512) ----
            for ch in range(B_CORE // 512):
                cs = slice(ch * 512, (ch + 1) * 512)
                hid_ps = ps_at.tile([128, 512], F32, tag="at", name="hid_ps")[:HIDDEN, :]
                for cc in range(2):
                    nc.tensor.matmul(hid_ps[:], fc1[:, cc, :], xgap[:, cc, cs],
                                     start=(cc == 0), stop=(cc == 1))
                hid = ppool.tile([HIDDEN, 512], F16, tag="hid_sb")
                nc.scalar.activation(hid[:], hid_ps[:], AF.Relu, bias=fc1b[:])
                log_ps = ps_at.tile([128, 512], F32, tag="at", name="log_ps")[:NK, :]
                nc.tensor.matmul(log_ps[:], fc2[:], hid[:], start=True, stop=True)
                e = ppool.tile([NK, 512], F16, tag="e")
                nc.scalar.activation(e[:], log_ps[:], AF.Exp, bias=fc2b[:],
                                     scale=float(1.0 / TEMP))
                S_ps = ps_at.tile([128, 512], F32, tag="at", name="S_ps")
                nc.tensor.matmul(S_ps[:], ones4[:], e[:], start=True, stop=True)
                recS = ppool.tile([128, 512], F32, tag="recS")
                nc.vector.reciprocal(recS[:], S_ps[:])
                nc.vector.tensor_mul(attn16[:, cs], e[:], recS[0:NK, :])
                for j, Em in enumerate((E01, E23)):
                    aps = ps_at.tile([128, 512], F32, tag="at", name=f"aE{j}")
                    nc.tensor.matmul(aps[:], Em[:], attn16[:, cs], start=True, stop=True)
                    nc.scalar.copy(attnE[:, j, cs], aps[:])

            # prefetch tile-1 x early (tile-0 x is preloaded outside the loop)
            for T in range(1, NTILES):
                ts = slice(T * NT, (T + 1) * NT)
                for t in range(4):
                    for cc in range(2):
                        nc.sync.dma_start(x_sb[cc][:, t, :, ts],
                                          dram["x_t"][cc, :, t, :, ts])

            featP = {}             # (T, ic) -> psum tile
            s_tiles = {}           # (T, ic, q) -> s plane tile
            kan_misc = {}

            def kan_elementwise(T, ic):
                m16 = work.tile([128, NT], F16, tag=f"m16_{ic}", name="m16")
                nc.vector.tensor_scalar(m16[:], featP[(T, ic)][:], 0.0, None, ALU.min)
                mp16 = work.tile([128, NT], F16, tag=f"mp16_{ic}", name="mp16")
                nc.scalar.activation(mp16[:], featP[(T, ic)][:], AF.Relu)
                sfeat = work.tile([128, NT], F16, tag=f"sf_{ic}", name="sfeat")
                nc.scalar.activation(sfeat[:], featP[(T, ic)][:], AF.Silu)
                ind = work.tile([128, NT], F16, tag=f"ind_{ic}", name="ind")
                nc.vector.tensor_scalar(ind[:], m16[:], 0.0, None, ALU.is_lt)
                r12 = rpool.tile([128, 12, NT], F32, tag="r12", name="r12")
                for half in range(2):
                    srct = m16 if half == 0 else mp16
                    clampop = ALU.max if half == 0 else ALU.min
                    for j in range(6):
                        q = half * 6 + j
                        nc.vector.tensor_scalar(
                            r12[:, q, :], srct[:], float(-G32[q]), 0.0,
                            ALU.add, clampop)
                r2b = rpool.tile([128, 12, NT], F16, tag="r2b", name="r2b")
                nc.scalar.activation(r2b[:], r12[:], AF.Square)
                for half in range(2):
                    srct = m16 if half == 0 else mp16
                    for j in range(6):
                        q = half * 6 + j
                        s = spool.tile([128, NT], F16, tag=f"s_{ic}_{q}", name="s")
                        # T0 left halves have long slack before their matmuls:
                        # compute s = r12*r2 on the idle GPSIMD engine there.
                        if T == 0 and half == 0:
                            nc.gpsimd.tensor_mul(s[:], r12[:, q, :], r2b[:, q, :])
                        else:
                            nc.vector.scalar_tensor_tensor(
                                s[:], srct[:], float(-G32[q]), r2b[:, q, :],
                                ALU.add, ALU.mult)
                        s_tiles[(T, ic, q)] = s
                kan_misc[(T, ic)] = (ind, sfeat)

            def conv_combine(T, oh):
                ts = slice(T * NT, (T + 1) * NT)
                fp = ps_f.tile([128, NT], F32, tag="fp", name=f"featP{oh}")
                featP[(T, oh)] = fp
                nmm = 0
                for och in range(2):
                    for t in range(4):
                        Y = ps_y.tile([128, NT], F32, tag="Y", name=f"Y{t}")
                        for cc in range(2):
                            for kh in range(3):
                                nc.tensor.matmul(
                                    Y[:],
                                    convW[:, t, kh, cc, och, :],
                                    x_sb[cc][:, t, oh + kh, ts],
                                    start=(cc == 0 and kh == 0),
                                    stop=(cc == 1 and kh == 2))
                        ysb = ypool.tile([128, NT], F16, tag="ysb", name="ysb")
                        nc.scalar.copy(ysb[:], Y[:])
                        tmp = ypool.tile([128, NT], F16, tag="tmp", name="tmp")
                        nc.vector.tensor_mul(tmp[:], ysb[:], attnE[:, och, ts])
                        nc.tensor.matmul(fp[:], A1sel[:, t, :], tmp[:],
                                         start=(nmm == 0), stop=False)
                        nmm += 1
                nc.tensor.matmul(fp[:], biasW[:], attn16[:, ts],
                                 start=False, stop=True)
                kan_elementwise(T, oh)

            def kan_matmuls(T):
                ts = slice(T * NT, (T + 1) * NT)
                out_ps = ps_o.tile([COUT, NT], F32, tag="out", name="out_ps")
                nmm = 0
                TOT_MM = 24 + 2 + 2 + 1
                for ic in range(2):
                    for q in range(12):
                        nc.tensor.matmul(out_ps[:], At[:, q, ic, :],
                                         s_tiles[(T, ic, q)][:],
                                         start=(nmm == 0), stop=(nmm == TOT_MM - 1))
                        nmm += 1
                    ind, sfeat = kan_misc[(T, ic)]
                    nc.tensor.matmul(out_ps[:], Aind[:, ic, :], ind[:],
                                     start=(nmm == 0), stop=(nmm == TOT_MM - 1)); nmm += 1
                    nc.tensor.matmul(out_ps[:], baseW[:, ic, :], sfeat[:],
                                     start=(nmm == 0), stop=(nmm == TOT_MM - 1)); nmm += 1
                nc.tensor.matmul(out_ps[:], C0row[:], ones2[:],
                                 start=False, stop=True); nmm += 1
                ob = opool.tile([COUT, NT], F32, tag="ob", name="ob")
                nc.scalar.copy(ob[:], out_ps[:])
                nc.sync.dma_start(out[:, ts], ob[:])

            # software-pipelined emission: T1 conv sits ahead of T0 KAN mms
            # in the PE queue so PE never waits on the spline DVE chain.
            conv_combine(0, 0)
            conv_combine(0, 1)
            conv_combine(1, 0)
            kan_matmuls(0)
            conv_combine(1, 1)
            kan_matmuls(1)

    nc.compile()
    return nc


def _get_compiled(reps=1):
    if ("nc", reps) not in _cached:
        _cached[("nc", reps)] = build_nc(reps)
    return _cached[("nc", reps)]


def _prep_x(x):
    """x [B, 256, 4, 4] fp32 -> per-core x_t [2,128,4t,4row,B_CORE] fp16 + xgap."""
    x = np.asarray(x, np.float32)
    xv = np.einsum("tw,bchw->bcth", BT_W, x)        # [B, 256, 4t, 4row]
    xr = xv.reshape(N_CORES, B_CORE, 2, 128, 4, 4)
    xt = np.ascontiguousarray(xr.transpose(0, 2, 3, 4, 5, 1)).astype(np.float16)
    g = x.mean(axis=(2, 3))                          # [B, 256]
    gr = g.reshape(N_CORES, B_CORE, 2, 128)
    gt = np.ascontiguousarray(gr.transpose(0, 2, 3, 1)).astype(np.float16)
    return xt, gt


def kernel(x, weight, bias, fc1_w, fc1_b, fc2_w, fc2_b,
           kan_base_w, kan_spline_w, kan_spline_scaler):
    wd = prepare_weights(weight, bias, fc1_w, fc1_b, fc2_w, fc2_b,
                         kan_base_w, kan_spline_w, kan_spline_scaler)
    nc = _get_compiled()
    xt, gt = _prep_x(x)
    in_maps = []
    for c in range(N_CORES):
        m = {"x_t": xt[c], "xgap": gt[c]}
        m.update(wd)
        in_maps.append(m)
    res = bass_utils.run_bass_kernel_spmd(nc, in_maps, core_ids=list(range(N_CORES)))
    out = np.concatenate([r["out"].T for r in res.results], axis=0)
    return out.astype(np.float32)


if __name__ == "__main__":
    sys.path.insert(0, "/root/problem")
    import reference as R
    inputs = {k: np.asarray(v) for k, v in R.setup_inputs().items()}
    got = kernel(**inputs)
    import jax
    with jax.default_device(jax.devices("cpu")[0]):
        exp = np.asarray(R.reference(**{k: jax.numpy.asarray(v) for k, v in inputs.items()}))
    rel = np.linalg.norm(got - exp) / np.linalg.norm(exp)
    print(f"Relative error: {rel:.3e}")


# revision 11
# speedup vs baseline: 1.8909x; 1.8909x over previous
"""Trainium2 Bass kernel for nn_CNN2D_48644799595070 (dynamic conv + attention + KAN).

Contract: kernel(**inputs) takes FULL unsharded inputs (np arrays keyed as in
setup_inputs) and returns the FULL [8192, 64] float32 output.  Internally:
batch is sharded over 8 NeuronCores (data parallel); all parameters are
replicated and host-folded into matmul-friendly fp16 tiles.

Math notes (device):
  conv:   1D Winograd F(2,3) along width.  Host ships V = x transformed per
          row (same byte count as x); device contracts 6 K-chunks (3 kh x 2
          cc) per (oh, t, och) Winograd-domain output: 96 matmuls per
          512-sample tile vs 144 direct.  Inverse transform (t -> ow,
          A = [[1,1,1,0],[0,1,-1,-1]]) is folded into the combine matmul.
  attn:   one-shot pre-pass over the whole core batch using a host-shipped
          GAP tile: fc1 -> relu -> fc2 -> exp(/T) -> recip-sum -> attn
          [4,B]; expanded to (kloc,oc) partitions via tiny matmuls and an
          ACT copy -> attnE fp16 in SBUF.
  combine: per (oh,t,och): ysb = ACT copy (PSUM->SBUF fp16); tmp = ysb *
          attnE (DVE fp16 2x); featP[oh] += A1sel[t]^T @ tmp (PSUM accum),
          + bias via attn16 matmul.
  KAN:    exact truncated-power spline: m16/mp16 = min/max(feat,0) fp16;
          r_q = (m16 - G_q) clamped (DVE dual-scalar TS, fp32 out); r2 =
          ACT Square batched 6 planes -> fp16; s_q = STT(m16,-G_q,r2) ->
          fp16; 24 plane matmuls + indicator + base silu + const row into
          one PSUM.
"""
import sys
sys.path.insert(0, "/opt/trn_rl_repo")

import numpy as np
from math import comb
from contextlib import ExitStack

import concourse.bass as bass
import concourse.tile as tile
from concourse import bacc, mybir
from concourse import bass_utils

# ---- problem constants (hardcoded per contract) ----
B_FULL = 8192
N_CORES = 8
B_CORE = B_FULL // N_CORES        # 1024
CIN = 256
COUT = 64
NK = 4
HIDDEN = 64
TEMP = 34.0
GRID_SIZE, SPLINE_ORDER = 5, 3
H = 0.4
G64 = np.arange(-SPLINE_ORDER, GRID_SIZE + SPLINE_ORDER + 1, dtype=np.float64) * H - 1.0
G32 = G64.astype(np.float32)

NT = 512          # b-tile (matmul moving free dim)
NTILES = B_CORE // NT

F32 = mybir.dt.float32
F16 = mybir.dt.float16
AF = mybir.ActivationFunctionType
ALU = mybir.AluOpType

# Winograd F(2,3) matrices
BT_W = np.array([[1, 0, -1, 0], [0, 1, 1, 0], [0, -1, 1, 0], [0, 1, 0, -1]], np.float32)
G1_W = np.array([[1, 0, 0], [.5, .5, .5], [.5, -.5, .5], [0, 0, 1]], np.float32)
A1_W = np.array([[1, 1, 1, 0], [0, 1, -1, -1]], np.float32)   # [ow, t]

_cached = {}


# --------------------------------------------------------------------------
# host-side weight folding
# --------------------------------------------------------------------------
def prepare_weights(weight, bias, fc1_w, fc1_b, fc2_w, fc2_b,
                    kan_base_w, kan_spline_w, kan_spline_scaler):
    """Fold all parameters into device tile layouts. Returns dict of np arrays."""
    d = {}
    f16 = np.float16
    # conv weights, Winograd domain: U[n,o,c,kh,t] = sum_kw G1[t,kw] w[n,o,c,kh,kw]
    w = np.asarray(weight, np.float32)                   # [NK, COUT, CIN, 3, 3]
    U = np.einsum("tw,nochw->nocht", G1_W, w).astype(f16)  # [NK,COUT,CIN,3,4]
    # lhsT tile per (t, kh, cc, och): [128 ci, 128 m=(kloc*64+oc)], k=och*2+kloc
    convW = np.empty((128, 4, 3, 2, 2, 128), f16)
    for t in range(4):
        for kh in range(3):
            for cc in range(2):
                for och in range(2):
                    blk = U[och * 2:och * 2 + 2, :, cc * 128:(cc + 1) * 128, kh, t]
                    # blk [2(kloc), 64(oc), 128(ci)] -> [ci, kloc*64+oc]
                    convW[:, t, kh, cc, och, :] = blk.reshape(128, 128).T
    d["convW"] = convW
    # combine lhsT per t: [128 (kloc,oc), 128 (ow*64+oc)] = A1[ow,t] * delta_oc
    A1sel = np.zeros((128, 4, 128), f16)
    for t in range(4):
        for p in range(128):
            oc = p % 64
            for ow in range(2):
                A1sel[p, t, ow * 64 + oc] = A1_W[ow, t]
    d["A1sel"] = A1sel
    # bias lhsT: [4 k, 128 (ow*64+oc)] = bias[k, oc]
    bW = np.zeros((4, 128), f16)
    bb = np.asarray(bias, np.float32)
    for k in range(4):
        for ow in range(2):
            bW[k, ow * 64:(ow + 1) * 64] = bb[k].astype(f16)
    d["biasW"] = bW
    # attention
    fc1 = np.asarray(fc1_w, np.float32)                  # xgap is the mean
    d["fc1"] = np.stack([fc1[:, cc * 128:(cc + 1) * 128].T.astype(f16)
                         for cc in range(2)], axis=1)    # [128, 2, 64]
    d["fc1b"] = np.asarray(fc1_b, np.float32).reshape(HIDDEN, 1)
    d["fc2"] = np.asarray(fc2_w, np.float32).T.astype(f16)    # [64, 4]
    d["fc2b34"] = (np.asarray(fc2_b, np.float32) / TEMP).reshape(NK, 1)
    E01 = np.zeros((4, 128), f16)
    E23 = np.zeros((4, 128), f16)
    for m in range(128):
        E01[m // 64, m] = 1.0
        E23[2 + m // 64, m] = 1.0
    d["E01"], d["E23"] = E01, E23

    # ---- KAN folding ----
    # device feat index i_new = po*64 + oc ; ref i = oc*4 + po
    i_new = np.arange(256)
    perm = (i_new % 64) * 4 + (i_new // 64)
    W2 = (np.asarray(kan_spline_w, np.float64)
          * np.asarray(kan_spline_scaler, np.float64)[..., None])   # [COUT,256,8]
    W2 = W2[:, perm, :]
    kbw = np.asarray(kan_base_w, np.float64)[:, perm]               # [COUT,256]

    c4 = np.array([comb(4, m) * (-1) ** m for m in range(5)], np.float64) / (6 * H ** 3)
    dd = np.zeros((COUT, 256, 12)); dp = np.zeros((COUT, 256, 12))
    for j in range(8):
        for m in range(5):
            dd[:, :, j + m] += W2[:, :, j] * c4[m]
            dp[:, :, j + 4 - m] += W2[:, :, j] * c4[m]
    A_L64 = dd[:, :, :6]           # left planes: s = (m-Gq)*relu(m-Gq)^2
    A_R64 = -dp[:, :, 6:]          # right planes: s = (mp-Gq)*relu(Gq-mp)^2 (= -r^3)

    def feedback_quant(A):
        Aq = np.empty_like(A)
        err = np.zeros(A.shape[:2])
        for q in range(A.shape[2]):
            t = A[:, :, q] + err
            Aq[:, :, q] = t.astype(f16).astype(np.float64)
            err = t - Aq[:, :, q]
        return Aq
    AL = feedback_quant(A_L64)
    AR = feedback_quant(A_R64[:, :, ::-1])[:, :, ::-1]

    # device-exact inactive plane values with the new op chain:
    #   left q (x>=0, m16=0):  r2 = f16(Gq^2); s = f16((0-Gq)*r2)
    #   right q (x<0, mp16=0): r2 = f16(Gq^2); s = f16((0-Gq)*r2)
    vL = np.empty(6, np.float64); wR = np.empty(6, np.float64)
    for q in range(6):
        r2 = np.float16(np.float32(-G32[q]) * np.float32(-G32[q]))
        vL[q] = np.float64(np.float16(np.float32(-G32[q]) * np.float32(r2)))
    for q in range(6, 12):
        r2 = np.float16(np.float32(-G32[q]) * np.float32(-G32[q]))
        wR[q - 6] = np.float64(np.float16(np.float32(-G32[q]) * np.float32(r2)))
    CposI = np.einsum("oiq,q->oi", AL, vL)
    CnegI = np.einsum("oiq,q->oi", AR, wR)
    Aind = (-(CnegI - CposI)).astype(f16)
    Cones = -CposI.sum(1)                      # [COUT] float64

    At = np.empty((128, 12, 2, 64), f16)       # [i_loc, q, ic, o]
    for q in range(12):
        srcq = AL[:, :, q] if q < 6 else AR[:, :, q - 6]   # [COUT, 256]
        for ic in range(2):
            At[:, q, ic, :] = srcq[:, ic * 128:(ic + 1) * 128].T.astype(f16)
    d["At"] = At
    d["Aind"] = np.stack([Aind[:, ic * 128:(ic + 1) * 128].T for ic in range(2)],
                         axis=1)                      # [128, 2, 64]
    d["baseW"] = np.stack([kbw[:, ic * 128:(ic + 1) * 128].T.astype(f16)
                           for ic in range(2)], axis=1)  # [128, 2, 64]
    C0hi = Cones.astype(f16)
    C0lo = (Cones - C0hi.astype(np.float64)).astype(f16)
    d["C0row"] = np.stack([C0hi, C0lo])          # [2, COUT] fp16
    return d


# --------------------------------------------------------------------------
# device kernel
# --------------------------------------------------------------------------
def build_nc(reps=1):
    nc = bacc.Bacc("TRN2", target_bir_lowering=False, debug=False,
                   enable_asserts=False, num_devices=N_CORES)
    dram = {}
    def din(name, shape, dt=F16):
        dram[name] = nc.dram_tensor(name, list(shape), dt, kind="ExternalInput").ap()
    din("x_t", (2, 128, 4, 4, B_CORE), F16)       # [cc, ci, t, row, b]
    din("xgap", (2, 128, B_CORE), F16)
    din("convW", (128, 4, 3, 2, 2, 128))
    din("A1sel", (128, 4, 128)); din("biasW", (4, 128))
    din("fc1", (128, 2, HIDDEN)); din("fc1b", (HIDDEN, 1), F32)
    din("fc2", (HIDDEN, NK)); din("fc2b34", (NK, 1), F32)
    din("E01", (4, 128)); din("E23", (4, 128))
    din("At", (128, 12, 2, COUT)); din("Aind", (128, 2, COUT))
    din("baseW", (128, 2, COUT)); din("C0row", (2, COUT))
    out = nc.dram_tensor("out", [COUT, B_CORE], F32, kind="ExternalOutput").ap()

    with tile.TileContext(nc) as tc, ExitStack() as ctx:
        wpool = ctx.enter_context(tc.tile_pool(name="weights", bufs=1))
        apool = ctx.enter_context(tc.tile_pool(name="attn", bufs=2))
        ppool = ctx.enter_context(tc.tile_pool(name="prepass", bufs=1))
        xpool = ctx.enter_context(tc.tile_pool(name="xdata", bufs=1))
        ypool = ctx.enter_context(tc.tile_pool(name="yevict", bufs=3))
        work = ctx.enter_context(tc.tile_pool(name="work", bufs=1))
        rpool = ctx.enter_context(tc.tile_pool(name="rpool", bufs=1))
        spool = ctx.enter_context(tc.tile_pool(name="splanes", bufs=1))
        opool = ctx.enter_context(tc.tile_pool(name="outbuf", bufs=2))
        ps_y = ctx.enter_context(tc.tile_pool(name="ps_y", bufs=3, space="PSUM"))
        ps_f = ctx.enter_context(tc.tile_pool(name="ps_feat", bufs=2, space="PSUM"))
        ps_o = ctx.enter_context(tc.tile_pool(name="ps_out", bufs=1, space="PSUM"))
        ps_at = ctx.enter_context(tc.tile_pool(name="ps_attn", bufs=1, space="PSUM"))

        # ---- DMA: attention inputs + weights first, then x tile 0 ----
        xgap = wpool.tile([128, 2, B_CORE], F16, name="xgap")
        nc.sync.dma_start(xgap[:, 0, :], dram["xgap"][0])
        nc.sync.dma_start(xgap[:, 1, :], dram["xgap"][1])
        fc1 = wpool.tile([128, 2, HIDDEN], F16); nc.sync.dma_start(fc1[:], dram["fc1"])
        fc1b = wpool.tile([HIDDEN, 1], F32); nc.sync.dma_start(fc1b[:], dram["fc1b"])
        fc2 = wpool.tile([HIDDEN, NK], F16); nc.sync.dma_start(fc2[:], dram["fc2"])
        fc2b = wpool.tile([NK, 1], F32); nc.sync.dma_start(fc2b[:], dram["fc2b34"])
        E01 = wpool.tile([4, 128], F16); nc.sync.dma_start(E01[:], dram["E01"])
        E23 = wpool.tile([4, 128], F16); nc.sync.dma_start(E23[:], dram["E23"])
        ones4 = wpool.tile([4, 128], F16); nc.any.memset(ones4[:], 1.0)
        ones2 = wpool.tile([2, NT], F16); nc.any.memset(ones2[:], 1.0)

        convW = wpool.tile([128, 4, 3, 2, 2, 128], F16)
        x_sb = []
        for cc in range(2):
            xc = xpool.tile([128, 4, 4, B_CORE], F16, tag=f"x{cc}", name=f"x_sb{cc}")
            x_sb.append(xc)
        # tile 0 x data + convW interleaved by t so conv can start early
        for t in range(4):
            nc.sync.dma_start(convW[:, t], dram["convW"][:, t])
            for cc in range(2):
                nc.sync.dma_start(x_sb[cc][:, t, :, 0:NT],
                                  dram["x_t"][cc, :, t, :, 0:NT])
        A1sel = wpool.tile([128, 4, 128], F16); nc.sync.dma_start(A1sel[:], dram["A1sel"])
        biasW = wpool.tile([4, 128], F16); nc.sync.dma_start(biasW[:], dram["biasW"])
        At = wpool.tile([128, 12, 2, COUT], F16); nc.sync.dma_start(At[:], dram["At"])
        Aind = wpool.tile([128, 2, COUT], F16); nc.sync.dma_start(Aind[:], dram["Aind"])
        baseW = wpool.tile([128, 2, COUT], F16); nc.sync.dma_start(baseW[:], dram["baseW"])
        C0row = wpool.tile([2, COUT], F16); nc.sync.dma_start(C0row[:], dram["C0row"])

        for _rep in range(reps):
            # per-rep attention outputs (double-buffered across reps)
            attnE = apool.tile([128, 2, B_CORE], F16, tag="attnE", name="attnE")
            attn16 = apool.tile([NK, B_CORE], F16, tag="attn16", name="attn16")
            # ---- attention pre-pass (full core batch, chunks of 512) ----
            for ch in range(B_CORE // 512):
                cs = slice(ch * 512, (ch + 1) * 512)
                hid_ps = ps_at.tile([128, 512], F32, tag="at", name="hid_ps")[:HIDDEN, :]
                for cc in range(2):
                    nc.tensor.matmul(hid_ps[:], fc1[:, cc, :], xgap[:, cc, cs],
                                     start=(cc == 0), stop=(cc == 1))
                hid = ppool.tile([HIDDEN, 512], F16, tag="hid_sb")
                nc.scalar.activation(hid[:], hid_ps[:], AF.Relu, bias=fc1b[:])
                log_ps = ps_at.tile([128, 512], F32, tag="at", name="log_ps")[:NK, :]
                nc.tensor.matmul(log_ps[:], fc2[:], hid[:], start=True, stop=True)
                e = ppool.tile([NK, 512], F16, tag="e")
                nc.scalar.activation(e[:], log_ps[:], AF.Exp, bias=fc2b[:],
                                     scale=float(1.0 / TEMP))
                S_ps = ps_at.tile([128, 512], F32, tag="at", name="S_ps")
                nc.tensor.matmul(S_ps[:], ones4[:], e[:], start=True, stop=True)
                recS = ppool.tile([128, 512], F32, tag="recS")
                nc.vector.reciprocal(recS[:], S_ps[:])
                nc.vector.tensor_mul(attn16[:, cs], e[:], recS[0:NK, :])
                for j, Em in enumerate((E01, E23)):
                    aps = ps_at.tile([128, 512], F32, tag="at", name=f"aE{j}")
                    nc.tensor.matmul(aps[:], Em[:], attn16[:, cs], start=True, stop=True)
                    nc.scalar.copy(attnE[:, j, cs], aps[:])

            # prefetch tile-1 x early (tile-0 x is preloaded outside the loop)
            for T in range(1, NTILES):
                ts = slice(T * NT, (T + 1) * NT)
                for t in range(4):
                    for cc in range(2):
                        nc.sync.dma_start(x_sb[cc][:, t, :, ts],
                                          dram["x_t"][cc, :, t, :, ts])

            featP = {}             # (T, ic) -> psum tile
            s_tiles = {}           # (T, ic, q) -> s plane tile
            kan_misc = {}

            def kan_elementwise(T, ic):
                m16 = work.tile([128, NT], F16, tag=f"m16_{ic}", name="m16")
                nc.vector.tensor_scalar(m16[:], featP[(T, ic)][:], 0.0, None, ALU.min)
                mp16 = work.tile([128, NT], F16, tag=f"mp16_{ic}", name="mp16")
                nc.scalar.activation(mp16[:], featP[(T, ic)][:], AF.Relu)
                sfeat = work.tile([128, NT], F16, tag=f"sf_{ic}", name="sfeat")
                nc.scalar.activation(sfeat[:], featP[(T, ic)][:], AF.Silu)
                ind = work.tile([128, NT], F16, tag=f"ind_{ic}", name="ind")
                nc.vector.tensor_scalar(ind[:], m16[:], 0.0, None, ALU.is_lt)
                r12 = rpool.tile([128, 12, NT], F32, tag="r12", name="r12")
                for half in range(2):
                    srct = m16 if half == 0 else mp16
                    clampop = ALU.max if half == 0 else ALU.min
                    for j in range(6):
                        q = half * 6 + j
                        nc.vector.tensor_scalar(
                            r12[:, q, :], srct[:], float(-G32[q]), 0.0,
                            ALU.add, clampop)
                r2b = rpool.tile([128, 12, NT], F16, tag="r2b", name="r2b")
                nc.scalar.activation(r2b[:], r12[:], AF.Square)
                for half in range(2):
                    srct = m16 if half == 0 else mp16
                    for j in range(6):
                        q = half * 6 + j
                        s = spool.tile([128, NT], F16, tag=f"s_{ic}_{q}", name="s")
                        nc.vector.scalar_tensor_tensor(
                            s[:], srct[:], float(-G32[q]), r2b[:, q, :],
                            ALU.add, ALU.mult)
                        s_tiles[(T, ic, q)] = s
                kan_misc[(T, ic)] = (ind, sfeat)

            def conv_combine(T, oh):
                ts = slice(T * NT, (T + 1) * NT)
                fp = ps_f.tile([128, NT], F32, tag="fp", name=f"featP{oh}")
                featP[(T, oh)] = fp
                nmm = 0
                for och in range(2):
                    for t in range(4):
                        Y = ps_y.tile([128, NT], F32, tag="Y", name=f"Y{t}")
                        for cc in range(2):
                            for kh in range(3):
                                nc.tensor.matmul(
                                    Y[:],
                                    convW[:, t, kh, cc, och, :],
                                    x_sb[cc][:, t, oh + kh, ts],
                                    start=(cc == 0 and kh == 0),
                                    stop=(cc == 1 and kh == 2))
                        ysb = ypool.tile([128, NT], F16, tag="ysb", name="ysb")
                        nc.scalar.copy(ysb[:], Y[:])
                        tmp = ypool.tile([128, NT], F16, tag="tmp", name="tmp")
                        nc.vector.tensor_mul(tmp[:], ysb[:], attnE[:, och, ts])
                        nc.tensor.matmul(fp[:], A1sel[:, t, :], tmp[:],
                                         start=(nmm == 0), stop=False)
                        nmm += 1
                nc.tensor.matmul(fp[:], biasW[:], attn16[:, ts],
                                 start=False, stop=True)
                kan_elementwise(T, oh)

            def kan_matmuls(T):
                ts = slice(T * NT, (T + 1) * NT)
                out_ps = ps_o.tile([COUT, NT], F32, tag="out", name="out_ps")
                nmm = 0
                TOT_MM = 24 + 2 + 2 + 1
                for ic in range(2):
                    for q in range(12):
                        nc.tensor.matmul(out_ps[:], At[:, q, ic, :],
                                         s_tiles[(T, ic, q)][:],
                                         start=(nmm == 0), stop=(nmm == TOT_MM - 1))
                        nmm += 1
                    ind, sfeat = kan_misc[(T, ic)]
                    nc.tensor.matmul(out_ps[:], Aind[:, ic, :], ind[:],
                                     start=(nmm == 0), stop=(nmm == TOT_MM - 1)); nmm += 1
                    nc.tensor.matmul(out_ps[:], baseW[:, ic, :], sfeat[:],
                                     start=(nmm == 0), stop=(nmm == TOT_MM - 1)); nmm += 1
                nc.tensor.matmul(out_ps[:], C0row[:], ones2[:],
                                 start=False, stop=True); nmm += 1
                ob = opool.tile([COUT, NT], F32, tag="ob", name="ob")
                nc.scalar.copy(ob[:], out_ps[:])
                nc.sync.dma_start(out[:, ts], ob[:])

            # software-pipelined emission: T1 conv sits ahead of T0 KAN mms
            # in the PE queue so PE never waits on the spline DVE chain.
            conv_combine(0, 0)
            conv_combine(0, 1)
            conv_combine(1, 0)
            kan_matmuls(0)
            conv_combine(1, 1)
            kan_matmuls(1)

    nc.compile()
    return nc


def _get_compiled(reps=1):
    if ("nc", reps) not in _cached:
        _cached[("nc", reps)] = build_nc(reps)
    return _cached[("nc", reps)]


def _prep_x(x):
    """x [B, 256, 4, 4] fp32 -> per-core x_t [2,128,4t,4row,B_CORE] fp16 + xgap."""
    x = np.asarray(x, np.float32)
    xv = np.einsum("tw,bchw->bcth", BT_W, x)        # [B, 256, 4t, 4row]
    xr = xv.reshape(N_CORES, B_CORE, 2, 128, 4, 4)
    xt = np.ascontiguousarray(xr.transpose(0, 2, 3, 4, 5, 1)).astype(np.float16)
    g = x.mean(axis=(2, 3))                          # [B, 256]
    gr = g.reshape(N_CORES, B_CORE, 2, 128)
    gt = np.ascontiguousarray(gr.transpose(0, 2, 3, 1)).astype(np.float16)
    return xt, gt


def kernel(x, weight, bias, fc1_w, fc1_b, fc2_w, fc2_b,
           kan_base_w, kan_spline_w, kan_spline_scaler):
    wd = prepare_weights(weight, bias, fc1_w, fc1_b, fc2_w, fc2_b,
                         kan_base_w, kan_spline_w, kan_spline_scaler)
    nc = _get_compiled()
    xt, gt = _prep_x(x)
    in_maps = []
    for c in range(N_CORES):
        m = {"x_t": xt[c], "xgap": gt[c]}
        m.update(wd)
        in_maps.append(m)
    res = bass_utils.run_bass_kernel_spmd(nc, in_maps, core_ids=list(range(N_CORES)))
    out = np.concatenate([r["out"].T for r in res.results], axis=0)
    return out.astype(np.float32)


if __name__ == "__main__":
    sys.path.insert(0, "/root/problem")
    import reference as R
    inputs = {k: np.asarray(v) for k, v in R.setup_inputs().items()}
    got = kernel(**inputs)
    import jax
    with jax.default_device(jax.devices("cpu")[0]):
        exp = np.asarray(R.reference(**{k: jax.numpy.asarray(v) for k, v in inputs.items()}))
    rel = np.linalg.norm(got - exp) / np.linalg.norm(exp)
    print(f"Relative error: {rel:.3e}")
